# revision 18
# baseline (speedup 1.0000x reference)
"""Trainium2 Bass kernel v2 for causal attention block (B=4, T=4096, D=256, k=v=64).

Geometry: 2 cores per batch; each core takes 8 q-chunks of 256 rows with causal
extents {8,7,6,5,4,3,2,1} (in 512-wide s-blocks): parity 0 gets even chunks
{14,12,...,0}, parity 1 odd {15,13,...,1}. All 8 cores run an IDENTICAL graph:
no dead blocks, and the diagonal (masked) block of every slot is statically the
last block. Per-core per-parity mask DATA (one [128,1024] fp8 pattern) handles
the even/odd-chunk triangle difference.

Pipeline per core:
  X^T folded fp8e4 (x8 host-scaled) -> K^T/Q^T via fp8 DoubleRow matmuls with
  folded weights (x32) -> PSUM f32 -> f32r SBUF (DVE/ACT copies).
  V natural via DoubleRow (stationary = X^T chunks) -> fp8e4 V_aug (x 1/64)
  with a fused ones column for the softmax denominator.
  Scores S^T [128s x 256q] in f32r (1 cyc/row), exp split between ACT
  (activation Exp -> fp8e5) and DVE (uint8 Schraudolph bit-pattern -> fp8e5);
  diagonal masks applied as a u32 bitwise-AND on DVE (00/FF byte masks).
  PV via mixed e4/e5 DoubleRow pair-matmuls accumulating [65 x 256] per slot
  (64 V rows + fused ones-column rowsum row). Host divides read/rowsum and
  scatters rows. Exp/drain work is statically cost-balanced across ACT+DVE
  with strict alternation; PV is deferred 5 tiles to hide exp latency.
"""

import math
import numpy as np
import ml_dtypes

B, T, D, KS = 4, 4096, 256, 64
CH = 256                      # q-chunk rows
SX, SW = 8.0, 32.0            # host scales for X and W (fp8 ranges)
SCALE_S = (SX * SW) ** 2      # score scale: 65536
Z_DEN = 8.0 * SCALE_S         # z = psum / Z_DEN
LOG2E = 1.4426950408889634
ACT_BIAS = -math.log(16.0)    # P' = exp(z)/16
AE = 4.0 * LOG2E / Z_DEN      # e5m2 pattern = 4*(log2(P') + 15) = 5.7708*z + 44
BE = 44.0                     # + sigma tuning term
SIGMA = 0.0
V_RESCALE = 1.0 / 64.0        # V' = 256*V/64 = 4*V
OUT_DIV = 4.0                 # host divides by this

_CACHE = {}


def _build():
    import concourse.bass as bass
    import concourse.tile as tile
    from concourse import bacc, mybir
    from contextlib import ExitStack
    from collections import deque

    f32 = mybir.dt.float32
    f32r = mybir.dt.float32r
    fp8e4 = mybir.dt.float8e4
    fp8e5 = mybir.dt.float8e5
    u8 = mybir.dt.uint8
    u32 = mybir.dt.uint32
    FT = mybir.ActivationFunctionType
    DR = mybir.MatmulPerfMode.DoubleRow
    AluOp = mybir.AluOpType
    MUL = mybir.AluOpType.mult
    ADD = mybir.AluOpType.add

    nc = bacc.Bacc("TRN2", target_bir_lowering=False, debug=False, num_devices=8)

    d_xf = nc.dram_tensor("xf", [128, 2 * T], fp8e4, kind="ExternalInput")
    d_xq = nc.dram_tensor("xq", [128, 4096], fp8e4, kind="ExternalInput")
    # wq | wk | wv | mask-bytes (mask is fp8e5 {0,1} patterns, shipped as bytes)
    d_cst = nc.dram_tensor("cst", [128, 3 * 128 + 1024], fp8e4,
                           kind="ExternalInput")
    d_out = nc.dram_tensor("out", [520, 256], f32, kind="ExternalOutput")

    # engine-balance bookkeeping (static assignment, costs in ns)
    bal = {"act": 0.0, "dve": 0.0}

    ACT_ADJ = 1.08  # measured ACT engine time runs ~8% over modeled

    def pick(cost_act, cost_dve):
        cost_act *= ACT_ADJ
        if bal["act"] + cost_act <= bal["dve"] + cost_dve:
            bal["act"] += cost_act
            return "act"
        bal["dve"] += cost_dve
        return "dve"

    with tile.TileContext(nc) as tc, ExitStack() as ctx:
        const = ctx.enter_context(tc.tile_pool(name="const", bufs=1))
        xp = ctx.enter_context(tc.tile_pool(name="xp", bufs=1))
        kqp = ctx.enter_context(tc.tile_pool(name="kqp", bufs=1))
        ptp = ctx.enter_context(tc.tile_pool(name="ptp", bufs=4))
        ovp = ctx.enter_context(tc.tile_pool(name="ovp", bufs=2))

        # ---- inputs first (xq pieces + consts unblock Q-proj; xf chases) ----
        xq_sb = xp.tile([128, 4096], fp8e4, name="xq")
        xq_f = xq_sb.rearrange("p (two t) -> p two t", two=2)
        d_xq_f = d_xq.ap().rearrange("p (two t) -> p two t", two=2)

        def dma_xq(p):
            nc.sync.dma_start(xq_f[:, :, 512 * p:512 * (p + 1)],
                              d_xq_f[:, :, 512 * p:512 * (p + 1)])

        cst_sb = const.tile([128, 3 * 128 + 1024], fp8e4, name="cst")
        bias_sb = const.tile([128, 1], f32, name="bias")

        xf_sb = xp.tile([128, 2 * T], fp8e4, name="xf")
        xf_f = xf_sb.rearrange("p (two t) -> p two t", two=2)
        d_xf_f = d_xf.ap().rearrange("p (two t) -> p two t", two=2)

        def dma_xf(j, eng=None):
            (eng or nc.sync).dma_start(xf_f[:, :, 1024 * j:1024 * (j + 1)],
                                       d_xf_f[:, :, 1024 * j:1024 * (j + 1)])

        dma_xq(0)
        nc.sync.dma_start(cst_sb[:], d_cst.ap())
        dma_xq(1)
        dma_xf(0)
        dma_xq(2)
        dma_xq(3)
        dma_xf(1)
        dma_xf(2)
        dma_xf(3)
        nc.vector.memset(bias_sb[:], ACT_BIAS)

        wq_f = cst_sb[:, 0:128].rearrange("p (two c) -> p two c", two=2)
        wk_f = cst_sb[:, 128:256].rearrange("p (two c) -> p two c", two=2)
        wv_f = cst_sb[:, 256:384].rearrange("p (two c) -> p two c", two=2)
        mask_u32 = cst_sb[:, 384:1408].bitcast(u32)

        # ---- projection targets ----
        k_sb = kqp.tile([64, T], f32r, name="ksb")
        q_sb = kqp.tile([64, 2048], f32r, name="qsb")
        v_aug = kqp.tile([128, 16 * 160], fp8e4, name="vaug")
        v_pairs = v_aug.rearrange("p (pr two c) -> p pr two c", two=2, c=80)
        nc.gpsimd.memset(v_pairs[:, :, :, 64:65], 1.0)

        ovs_sbuf = {}

        with tc.tile_pool(name="ringp", bufs=3, space="PSUM") as ringp, \
             tc.tile_pool(name="pvp", bufs=2, space="PSUM") as pvp:

            def drain_copy(dst, src, n, scale=None, eng=None):
                """PSUM->SBUF drain, engine-balanced. n = free size."""
                if eng is None:
                    eng = pick(n * 0.8333 + 143.0, n * 1.0417 + 125.0)
                else:
                    bal[eng] += ((n * 0.8333 + 143.0) * ACT_ADJ if eng == "act"
                                 else n * 1.0417 + 125.0)
                if eng == "act":
                    nc.scalar.activation(dst, src, FT.Copy,
                                         scale=1.0 if scale is None else scale)
                elif scale is None:
                    nc.vector.tensor_copy(dst, src)
                else:
                    nc.vector.tensor_scalar_mul(dst, src, scale)

            # ---- emit helpers ----
            pending = deque()
            state = {"expi": 0, "pv_first": {}}

            def emit_pv(item):
                pt8, s, b, first, last = item
                if first:
                    ovs_sbuf[s] = pvp.tile([65, 256], f32, name="ov", tag="pv")
                ov = ovs_sbuf[s]
                for i in range(2):
                    pt_pair = pt8[:, 512 * i:512 * (i + 1)].rearrange(
                        "p (two q) -> p two q", two=2)
                    nc.tensor.matmul(
                        ov[:], v_pairs[:, 2 * b + i, :, 0:65], pt_pair,
                        start=(first and i == 0),
                        stop=(last and i == 1), perf_mode=DR)
                if last:
                    fin = ptp.tile([128, 256], f32, name="fin", tag="fin",
                                   bufs=4)[0:65, :]
                    drain_copy(fin, ov[:], 256)
                    nc.sync.dma_start(d_out.ap()[65 * s:65 * (s + 1), :], fin)

            def emit_tile(s, b, first, last, diag):
                rg = ringp.tile([128, 1024], f32, name="rg", tag="ring")
                for k in range(4):
                    sub = 4 * b + k
                    nc.tensor.matmul(
                        rg[:, 256 * k:256 * (k + 1)],
                        k_sb[:, 128 * sub:128 * (sub + 1)],
                        q_sb[:, 256 * s:256 * (s + 1)],
                        start=(k % 2 == 0), stop=(k % 2 == 1))
                # alternate exp engines (keeps both pipelines fed); fall back
                # to the lighter engine only on heavy imbalance
                ca = (1024 * 0.8333 + 143.0) * ACT_ADJ
                cd = 1024 * 1.0417 + 125.0
                eng = "act" if state.get("last_exp") == "dve" else "dve"
                if eng == "act" and bal["act"] + ca > bal["dve"] + cd + 600.0:
                    eng = "dve"
                elif eng == "dve" and bal["dve"] + cd > bal["act"] + ca + 600.0:
                    eng = "act"
                bal[eng] += ca if eng == "act" else cd
                state["last_exp"] = eng
                state["expi"] += 1
                if eng == "act":
                    pt8 = ptp.tile([128, 1024], fp8e5, name="pt8", tag="pt",
                                   bufs=8)
                    nc.scalar.activation(pt8[:], rg[:], FT.Exp,
                                         bias=bias_sb[:], scale=1.0 / Z_DEN)
                else:
                    ptu = ptp.tile([128, 1024], u8, name="ptu", tag="pt",
                                   bufs=8)
                    nc.vector.tensor_scalar(ptu[:], rg[:], AE, BE + SIGMA,
                                            MUL, ADD)
                    pt8 = ptu.bitcast(fp8e5)
                if diag:
                    # {0,1} mask multiply == bitwise AND with 00/FF bytes;
                    # u32 view cuts the ap by 4x (bitwise ops are DVE-only)
                    bal["dve"] += 330.0
                    nc.vector.tensor_tensor(pt8.bitcast(u32)[:],
                                            pt8.bitcast(u32)[:],
                                            mask_u32[:], AluOp.bitwise_and)
                pending.append((pt8, s, b, first, last))
                if len(pending) > 5:
                    emit_pv(pending.popleft())

            # ---- projection emitters ----
            def emit_qpack(i, eng=None):
                psq = ringp.tile([128, 1024], f32, name="psq", tag="ring")
                for ii in range(4):
                    s = 4 * i + ii
                    nc.tensor.matmul(
                        psq[0:64, 256 * ii:256 * (ii + 1)],
                        wq_f, xq_f[:, :, 256 * s:256 * (s + 1)],
                        start=(ii % 2 == 0), stop=(ii % 2 == 1), perf_mode=DR)
                drain_copy(q_sb[:, 1024 * i:1024 * (i + 1)], psq[0:64, :],
                           1024, eng=eng)

            def emit_kpack(kp, eng=None):
                psk = ringp.tile([128, 1024], f32, name="psk", tag="ring")
                for ii in range(2):
                    w = 2 * kp + ii
                    nc.tensor.matmul(
                        psk[0:64, 512 * ii:512 * (ii + 1)],
                        wk_f, xf_f[:, :, 512 * w:512 * (w + 1)],
                        start=True, stop=True, perf_mode=DR)
                drain_copy(k_sb[:, 1024 * kp:1024 * (kp + 1)], psk[0:64, :],
                           1024, eng=eng)

            def emit_vpack(m):
                psv = ringp.tile([128, 1024], f32, name="psv", tag="ring")
                for jj in range(16):
                    sub = 16 * m + jj
                    nc.tensor.matmul(
                        psv[:, 64 * jj:64 * (jj + 1)],
                        xf_f[:, :, 128 * sub:128 * (sub + 1)], wv_f,
                        start=(jj % 8 == 0), stop=(jj % 8 == 7), perf_mode=DR)
                drain_copy(
                    v_pairs[:, 8 * m:8 * (m + 1), :, 0:64],
                    psv[:, :].rearrange("p (pr two c) -> p pr two c",
                                        two=2, c=64),
                    1024, scale=V_RESCALE)

            def t0(b):
                emit_tile(0, b, first=(b == 0), last=(b == 7), diag=(b == 7))

            # projections first (they own the ring rotation), slot-0 tiles
            # after each half so exp engines fill as soon as data lands
            emit_qpack(0)
            emit_qpack(1)
            emit_kpack(0)
            emit_kpack(1)
            emit_vpack(0)
            for b in range(4):
                t0(b)
            emit_kpack(2)
            emit_kpack(3)
            emit_vpack(1)
            for b in range(4, 8):
                t0(b)

            # ---- remaining slots: interleave tiles from slot PAIRS (two
            # live PV accumulators = the two pvp bufs) so slot-boundary
            # dependency chains overlap across both exp engines ----
            def emit_group(slots, taper=False):
                streams = []
                for s in slots:
                    v = 8 - s
                    order = [v - 1] + list(range(v - 1))
                    streams.append([(s, b, i == 0, i == v - 1, b == v - 1)
                                    for i, b in enumerate(order)])
                while any(streams):
                    for st in streams:
                        if st:
                            s, b, first, last, diag = st.pop(0)
                            emit_tile(s, b, first=first, last=last, diag=diag)
                if taper:
                    while len(pending) > 2:
                        emit_pv(pending.popleft())

            emit_group([1, 7])
            emit_group([2, 6])
            emit_group([3, 5], taper=True)
            emit_group([4], taper=True)
            while pending:
                emit_pv(pending.popleft())

    nc.compile()
    return nc


def _host_prep():
    """Per-parity diag byte-masks (0xFF keep / 0x00 kill, ANDed onto fp8 P)."""
    p = np.arange(128)[:, None]
    q = np.arange(256)[None, :]
    tri0 = np.where(p <= q, 0xFF, 0).astype(np.uint8)
    tri1 = np.where(p + 128 <= q, 0xFF, 0).astype(np.uint8)
    ones = np.full((128, 256), 0xFF, np.uint8)
    zeros = np.zeros((128, 256), np.uint8)
    m0 = np.concatenate([tri0, tri1, zeros, zeros], axis=1)
    m1 = np.concatenate([ones, ones, tri0, tri1], axis=1)
    return m0, m1


def _get_nc():
    if "nc" not in _CACHE:
        _CACHE["nc"] = _build()
    return _CACHE["nc"]


def kernel(inputs, key_w, query_w, value_w):
    from concourse.bass_utils import run_bass_kernel_spmd

    e4 = ml_dtypes.float8_e4m3
    inputs = np.asarray(inputs, np.float32)

    def fold_w(w):
        ws = (np.asarray(w, np.float32) * SW).astype(e4)
        return np.ascontiguousarray(np.concatenate([ws[0:128], ws[128:256]],
                                                   axis=1))  # [128, 128]

    wq8, wk8, wv8 = fold_w(query_w), fold_w(key_w), fold_w(value_w)
    mask0, mask1 = _host_prep()
    w_part = np.concatenate([wq8, wk8, wv8], axis=1)  # [128, 384] e4m3
    csts = {
        par: np.ascontiguousarray(np.concatenate(
            [w_part.view(np.uint8), m], axis=1).view(e4))
        for par, m in ((0, mask0), (1, mask1))
    }

    in_maps = []
    chunk_ids = {}
    for c in range(8):
        b, par = c // 2, c % 2
        xT = (inputs[b].T * SX).astype(e4)          # [256, 4096]
        xf = np.ascontiguousarray(np.concatenate([xT[0:128], xT[128:256]],
                                                 axis=1))  # [128, 8192]
        chs = [(14 - 2 * s if par == 0 else 15 - 2 * s) for s in range(8)]
        chunk_ids[c] = chs
        xq_half0 = np.concatenate([xT[0:128, CH * ch:CH * (ch + 1)]
                                   for ch in chs], axis=1)   # [128, 2048]
        xq_half1 = np.concatenate([xT[128:256, CH * ch:CH * (ch + 1)]
                                   for ch in chs], axis=1)
        xq = np.ascontiguousarray(np.concatenate([xq_half0, xq_half1], axis=1))
        in_maps.append({"xf": xf, "xq": xq, "cst": csts[par]})

    nc = _get_nc()
    res = run_bass_kernel_spmd(nc, in_maps, core_ids=list(range(8))).results

    out = np.empty((B, T, D + KS), np.float32)
    out[:, :, :D] = inputs
    for c in range(8):
        b = c // 2
        r = np.asarray(res[c]["out"] if isinstance(res[c], dict) else res[c],
                       np.float32)  # [520, 256]
        for s in range(8):
            ch = chunk_ids[c][s]
            blkr = r[65 * s:65 * (s + 1)]          # [65, 256]
            read = blkr[0:64].T                     # [256, 64]
            rowsum = blkr[64]                       # [256]
            rowsum = np.where(rowsum == 0.0, 1.0, rowsum)
            out[b, CH * ch:CH * (ch + 1), D:] = read / rowsum[:, None] / OUT_DIV
    return out


# revision 19
# speedup vs baseline: 1.0064x; 1.0064x over previous
"""Trainium2 Bass kernel v2 for causal attention block (B=4, T=4096, D=256, k=v=64).

Geometry: 2 cores per batch; each core takes 8 q-chunks of 256 rows with causal
extents {8,7,6,5,4,3,2,1} (in 512-wide s-blocks): parity 0 gets even chunks
{14,12,...,0}, parity 1 odd {15,13,...,1}. All 8 cores run an IDENTICAL graph:
no dead blocks, and the diagonal (masked) block of every slot is statically the
last block. Per-core per-parity mask DATA (one [128,1024] fp8 pattern) handles
the even/odd-chunk triangle difference.

Pipeline per core:
  X^T folded fp8e4 (x8 host-scaled) -> K^T/Q^T via fp8 DoubleRow matmuls with
  folded weights (x32) -> PSUM f32 -> f32r SBUF (DVE/ACT copies).
  V natural via DoubleRow (stationary = X^T chunks) -> fp8e4 V_aug (x 1/64)
  with a fused ones column for the softmax denominator.
  Scores S^T [128s x 256q] in f32r (1 cyc/row), exp split between ACT
  (activation Exp -> fp8e5) and DVE (uint8 Schraudolph bit-pattern -> fp8e5);
  diagonal masks applied as a u32 bitwise-AND on DVE (00/FF byte masks).
  PV via mixed e4/e5 DoubleRow pair-matmuls accumulating [65 x 256] per slot
  (64 V rows + fused ones-column rowsum row). Host divides read/rowsum and
  scatters rows. Exp/drain work is statically cost-balanced across ACT+DVE
  with strict alternation; PV is deferred 5 tiles to hide exp latency.
"""

import math
import numpy as np
import ml_dtypes

B, T, D, KS = 4, 4096, 256, 64
CH = 256                      # q-chunk rows
SX, SW = 8.0, 32.0            # host scales for X and W (fp8 ranges)
SCALE_S = (SX * SW) ** 2      # score scale: 65536
Z_DEN = 8.0 * SCALE_S         # z = psum / Z_DEN
LOG2E = 1.4426950408889634
ACT_BIAS = -math.log(16.0)    # P' = exp(z)/16
AE = 4.0 * LOG2E / Z_DEN      # e5m2 pattern = 4*(log2(P') + 15) = 5.7708*z + 44
BE = 44.0                     # + sigma tuning term
SIGMA = 0.0
V_RESCALE = 1.0 / 64.0        # V' = 256*V/64 = 4*V
OUT_DIV = 4.0                 # host divides by this

_CACHE = {}


def _build():
    import concourse.bass as bass
    import concourse.tile as tile
    from concourse import bacc, mybir
    from contextlib import ExitStack
    from collections import deque

    f32 = mybir.dt.float32
    f32r = mybir.dt.float32r
    fp8e4 = mybir.dt.float8e4
    fp8e5 = mybir.dt.float8e5
    u8 = mybir.dt.uint8
    u32 = mybir.dt.uint32
    FT = mybir.ActivationFunctionType
    DR = mybir.MatmulPerfMode.DoubleRow
    AluOp = mybir.AluOpType
    MUL = mybir.AluOpType.mult
    ADD = mybir.AluOpType.add

    nc = bacc.Bacc("TRN2", target_bir_lowering=False, debug=False, num_devices=8)

    d_xf = nc.dram_tensor("xf", [128, 2 * T], fp8e4, kind="ExternalInput")
    d_xq = nc.dram_tensor("xq", [128, 4096], fp8e4, kind="ExternalInput")
    # wq | wk | wv | mask-bytes (mask is fp8e5 {0,1} patterns, shipped as bytes)
    d_cst = nc.dram_tensor("cst", [128, 3 * 128 + 1024], fp8e4,
                           kind="ExternalInput")
    d_out = nc.dram_tensor("out", [520, 256], f32, kind="ExternalOutput")

    # engine-balance bookkeeping (static assignment, costs in ns)
    bal = {"act": 0.0, "dve": 0.0}

    ACT_ADJ = 1.08  # measured ACT engine time runs ~8% over modeled

    def pick(cost_act, cost_dve):
        cost_act *= ACT_ADJ
        if bal["act"] + cost_act <= bal["dve"] + cost_dve:
            bal["act"] += cost_act
            return "act"
        bal["dve"] += cost_dve
        return "dve"

    with tile.TileContext(nc) as tc, ExitStack() as ctx:
        const = ctx.enter_context(tc.tile_pool(name="const", bufs=1))
        xp = ctx.enter_context(tc.tile_pool(name="xp", bufs=1))
        kqp = ctx.enter_context(tc.tile_pool(name="kqp", bufs=1))
        ptp = ctx.enter_context(tc.tile_pool(name="ptp", bufs=4))
        ovp = ctx.enter_context(tc.tile_pool(name="ovp", bufs=2))

        # ---- inputs first (xq pieces + consts unblock Q-proj; xf chases) ----
        xq_sb = xp.tile([128, 4096], fp8e4, name="xq")
        xq_f = xq_sb.rearrange("p (two t) -> p two t", two=2)
        d_xq_f = d_xq.ap().rearrange("p (two t) -> p two t", two=2)

        def dma_xq(p):
            nc.sync.dma_start(xq_f[:, :, 512 * p:512 * (p + 1)],
                              d_xq_f[:, :, 512 * p:512 * (p + 1)])

        cst_sb = const.tile([128, 3 * 128 + 1024], fp8e4, name="cst")
        bias_sb = const.tile([128, 1], f32, name="bias")

        xf_sb = xp.tile([128, 2 * T], fp8e4, name="xf")
        xf_f = xf_sb.rearrange("p (two t) -> p two t", two=2)
        d_xf_f = d_xf.ap().rearrange("p (two t) -> p two t", two=2)

        def dma_xf(j, eng=None):
            (eng or nc.sync).dma_start(xf_f[:, :, 1024 * j:1024 * (j + 1)],
                                       d_xf_f[:, :, 1024 * j:1024 * (j + 1)])

        dma_xq(0)
        nc.sync.dma_start(cst_sb[:], d_cst.ap())
        dma_xq(1)
        dma_xf(0)
        dma_xq(2)
        dma_xq(3)
        dma_xf(1)
        dma_xf(2)
        dma_xf(3)
        nc.vector.memset(bias_sb[:], ACT_BIAS)

        wq_f = cst_sb[:, 0:128].rearrange("p (two c) -> p two c", two=2)
        wk_f = cst_sb[:, 128:256].rearrange("p (two c) -> p two c", two=2)
        wv_f = cst_sb[:, 256:384].rearrange("p (two c) -> p two c", two=2)
        mask_u32 = cst_sb[:, 384:1408].bitcast(u32)

        # ---- projection targets ----
        k_sb = kqp.tile([64, T], f32r, name="ksb")
        q_sb = kqp.tile([64, 2048], f32r, name="qsb")
        v_aug = kqp.tile([128, 16 * 160], fp8e4, name="vaug")
        v_pairs = v_aug.rearrange("p (pr two c) -> p pr two c", two=2, c=80)
        nc.gpsimd.memset(v_pairs[:, :, :, 64:65], 1.0)

        ovs_sbuf = {}

        with tc.tile_pool(name="ringp", bufs=3, space="PSUM") as ringp, \
             tc.tile_pool(name="pvp", bufs=2, space="PSUM") as pvp:

            def drain_copy(dst, src, n, scale=None, eng=None):
                """PSUM->SBUF drain, engine-balanced. n = free size."""
                if eng is None:
                    eng = pick(n * 0.8333 + 143.0, n * 1.0417 + 125.0)
                else:
                    bal[eng] += ((n * 0.8333 + 143.0) * ACT_ADJ if eng == "act"
                                 else n * 1.0417 + 125.0)
                if eng == "act":
                    nc.scalar.activation(dst, src, FT.Copy,
                                         scale=1.0 if scale is None else scale)
                elif scale is None:
                    nc.vector.tensor_copy(dst, src)
                else:
                    nc.vector.tensor_scalar_mul(dst, src, scale)

            # ---- emit helpers ----
            pending = deque()
            state = {"expi": 0, "pv_first": {}}

            def emit_pv(item):
                pt8, s, b, first, last = item
                if first:
                    ovs_sbuf[s] = pvp.tile([65, 256], f32, name="ov", tag="pv")
                ov = ovs_sbuf[s]
                for i in range(2):
                    pt_pair = pt8[:, 512 * i:512 * (i + 1)].rearrange(
                        "p (two q) -> p two q", two=2)
                    nc.tensor.matmul(
                        ov[:], v_pairs[:, 2 * b + i, :, 0:65], pt_pair,
                        start=(first and i == 0),
                        stop=(last and i == 1), perf_mode=DR)
                if last:
                    fin = ptp.tile([128, 256], f32, name="fin", tag="fin",
                                   bufs=4)[0:65, :]
                    drain_copy(fin, ov[:], 256)
                    nc.sync.dma_start(d_out.ap()[65 * s:65 * (s + 1), :], fin)

            def emit_tile(s, b, first, last, diag):
                rg = ringp.tile([128, 1024], f32, name="rg", tag="ring")
                for k in range(4):
                    sub = 4 * b + k
                    nc.tensor.matmul(
                        rg[:, 256 * k:256 * (k + 1)],
                        k_sb[:, 128 * sub:128 * (sub + 1)],
                        q_sb[:, 256 * s:256 * (s + 1)],
                        start=(k % 2 == 0), stop=(k % 2 == 1))
                # alternate exp engines (keeps both pipelines fed); fall back
                # to the lighter engine only on heavy imbalance
                ca = (1024 * 0.8333 + 143.0) * ACT_ADJ
                cd = 1024 * 1.0417 + 125.0
                eng = "act" if state.get("last_exp") == "dve" else "dve"
                if eng == "act" and bal["act"] + ca > bal["dve"] + cd + 600.0:
                    eng = "dve"
                elif eng == "dve" and bal["dve"] + cd > bal["act"] + ca + 600.0:
                    eng = "act"
                bal[eng] += ca if eng == "act" else cd
                state["last_exp"] = eng
                state["expi"] += 1
                if eng == "act":
                    pt8 = ptp.tile([128, 1024], fp8e5, name="pt8", tag="pt",
                                   bufs=8)
                    nc.scalar.activation(pt8[:], rg[:], FT.Exp,
                                         bias=bias_sb[:], scale=1.0 / Z_DEN)
                else:
                    ptu = ptp.tile([128, 1024], u8, name="ptu", tag="pt",
                                   bufs=8)
                    nc.vector.tensor_scalar(ptu[:], rg[:], AE, BE + SIGMA,
                                            MUL, ADD)
                    pt8 = ptu.bitcast(fp8e5)
                if diag:
                    # {0,1} mask multiply == bitwise AND with 00/FF bytes;
                    # u32 view cuts the ap by 4x (bitwise ops are DVE-only)
                    bal["dve"] += 330.0
                    nc.vector.tensor_tensor(pt8.bitcast(u32)[:],
                                            pt8.bitcast(u32)[:],
                                            mask_u32[:], AluOp.bitwise_and)
                pending.append((pt8, s, b, first, last))
                if len(pending) > 5:
                    emit_pv(pending.popleft())

            # ---- projection emitters ----
            def emit_qpack(i, eng=None):
                psq = ringp.tile([128, 1024], f32, name="psq", tag="ring")
                for ii in range(4):
                    s = 4 * i + ii
                    nc.tensor.matmul(
                        psq[0:64, 256 * ii:256 * (ii + 1)],
                        wq_f, xq_f[:, :, 256 * s:256 * (s + 1)],
                        start=(ii % 2 == 0), stop=(ii % 2 == 1), perf_mode=DR)
                drain_copy(q_sb[:, 1024 * i:1024 * (i + 1)], psq[0:64, :],
                           1024, eng=eng)

            def emit_kpack(kp, eng=None):
                psk = ringp.tile([128, 1024], f32, name="psk", tag="ring")
                for ii in range(2):
                    w = 2 * kp + ii
                    nc.tensor.matmul(
                        psk[0:64, 512 * ii:512 * (ii + 1)],
                        wk_f, xf_f[:, :, 512 * w:512 * (w + 1)],
                        start=True, stop=True, perf_mode=DR)
                drain_copy(k_sb[:, 1024 * kp:1024 * (kp + 1)], psk[0:64, :],
                           1024, eng=eng)

            def emit_vpack(m):
                psv = ringp.tile([128, 1024], f32, name="psv", tag="ring")
                for jj in range(16):
                    sub = 16 * m + jj
                    nc.tensor.matmul(
                        psv[:, 64 * jj:64 * (jj + 1)],
                        xf_f[:, :, 128 * sub:128 * (sub + 1)], wv_f,
                        start=(jj % 8 == 0), stop=(jj % 8 == 7), perf_mode=DR)
                drain_copy(
                    v_pairs[:, 8 * m:8 * (m + 1), :, 0:64],
                    psv[:, :].rearrange("p (pr two c) -> p pr two c",
                                        two=2, c=64),
                    1024, scale=V_RESCALE)

            def t0(b):
                emit_tile(0, b, first=(b == 0), last=(b == 7), diag=(b == 7))

            # projections first (they own the ring rotation), slot-0 tiles
            # after each half so exp engines fill as soon as data lands
            emit_qpack(0)
            emit_qpack(1)
            emit_kpack(0)
            emit_kpack(1)
            emit_vpack(0)
            for b in range(4):
                t0(b)
            emit_kpack(2)
            emit_kpack(3)
            emit_vpack(1)
            for b in range(4, 8):
                t0(b)

            # ---- remaining slots: interleave tiles from slot PAIRS (two
            # live PV accumulators = the two pvp bufs) so slot-boundary
            # dependency chains overlap across both exp engines ----
            def emit_group(slots, taper=False):
                streams = []
                for s in slots:
                    v = 8 - s
                    order = [v - 1] + list(range(v - 1))
                    streams.append([(s, b, i == 0, i == v - 1, b == v - 1)
                                    for i, b in enumerate(order)])
                while any(streams):
                    for st in streams:
                        if st:
                            s, b, first, last, diag = st.pop(0)
                            emit_tile(s, b, first=first, last=last, diag=diag)
                if taper:
                    while len(pending) > 2:
                        emit_pv(pending.popleft())

            emit_group([1, 2])
            emit_group([7, 3])
            emit_group([6, 4], taper=True)
            emit_group([5], taper=True)
            while pending:
                emit_pv(pending.popleft())

    nc.compile()
    return nc


def _host_prep():
    """Per-parity diag byte-masks (0xFF keep / 0x00 kill, ANDed onto fp8 P)."""
    p = np.arange(128)[:, None]
    q = np.arange(256)[None, :]
    tri0 = np.where(p <= q, 0xFF, 0).astype(np.uint8)
    tri1 = np.where(p + 128 <= q, 0xFF, 0).astype(np.uint8)
    ones = np.full((128, 256), 0xFF, np.uint8)
    zeros = np.zeros((128, 256), np.uint8)
    m0 = np.concatenate([tri0, tri1, zeros, zeros], axis=1)
    m1 = np.concatenate([ones, ones, tri0, tri1], axis=1)
    return m0, m1


def _get_nc():
    if "nc" not in _CACHE:
        _CACHE["nc"] = _build()
    return _CACHE["nc"]


def kernel(inputs, key_w, query_w, value_w):
    from concourse.bass_utils import run_bass_kernel_spmd

    e4 = ml_dtypes.float8_e4m3
    inputs = np.asarray(inputs, np.float32)

    def fold_w(w):
        ws = (np.asarray(w, np.float32) * SW).astype(e4)
        return np.ascontiguousarray(np.concatenate([ws[0:128], ws[128:256]],
                                                   axis=1))  # [128, 128]

    wq8, wk8, wv8 = fold_w(query_w), fold_w(key_w), fold_w(value_w)
    mask0, mask1 = _host_prep()
    w_part = np.concatenate([wq8, wk8, wv8], axis=1)  # [128, 384] e4m3
    csts = {
        par: np.ascontiguousarray(np.concatenate(
            [w_part.view(np.uint8), m], axis=1).view(e4))
        for par, m in ((0, mask0), (1, mask1))
    }

    in_maps = []
    chunk_ids = {}
    for c in range(8):
        b, par = c // 2, c % 2
        xT = (inputs[b].T * SX).astype(e4)          # [256, 4096]
        xf = np.ascontiguousarray(np.concatenate([xT[0:128], xT[128:256]],
                                                 axis=1))  # [128, 8192]
        chs = [(14 - 2 * s if par == 0 else 15 - 2 * s) for s in range(8)]
        chunk_ids[c] = chs
        xq_half0 = np.concatenate([xT[0:128, CH * ch:CH * (ch + 1)]
                                   for ch in chs], axis=1)   # [128, 2048]
        xq_half1 = np.concatenate([xT[128:256, CH * ch:CH * (ch + 1)]
                                   for ch in chs], axis=1)
        xq = np.ascontiguousarray(np.concatenate([xq_half0, xq_half1], axis=1))
        in_maps.append({"xf": xf, "xq": xq, "cst": csts[par]})

    nc = _get_nc()
    res = run_bass_kernel_spmd(nc, in_maps, core_ids=list(range(8))).results

    out = np.empty((B, T, D + KS), np.float32)
    out[:, :, :D] = inputs
    for c in range(8):
        b = c // 2
        r = np.asarray(res[c]["out"] if isinstance(res[c], dict) else res[c],
                       np.float32)  # [520, 256]
        for s in range(8):
            ch = chunk_ids[c][s]
            blkr = r[65 * s:65 * (s + 1)]          # [65, 256]
            read = blkr[0:64].T                     # [256, 64]
            rowsum = blkr[64]                       # [256]
            rowsum = np.where(rowsum == 0.0, 1.0, rowsum)
            out[b, CH * ch:CH * (ch + 1), D:] = read / rowsum[:, None] / OUT_DIV
    return out
